# revision 3
# baseline (speedup 1.0000x reference)
"""DeepseekV3 MoE (T=1024, D=2048, E=32, grouped top-4 routing, I=1408,
shared expert 2816) on 8 trn2 NeuronCores via Bass/Tile.

Expert-parallel sparse dispatch; see build_program for the device-side
structure. Host computes the gate forward once (numpy) ONLY to size
per-expert capacities and balance the expert->(core,slot) assignment;
all model math runs on device.
"""

import numpy as np
import ml_dtypes
from einops import rearrange

import concourse.bass as bass
import concourse.bacc as bacc
import concourse.mybir as mybir
from concourse.tile import TileContext
from concourse.bass_utils import run_bass_kernel_spmd

F32 = mybir.dt.float32
BF16 = mybir.dt.bfloat16
I32 = mybir.dt.int32

T, D, E, I = 1024, 2048, 32, 1408
G, TOPK_GROUP, TOP_K = 8, 4, 4
SHARED_I = 2816
ROUTED_SCALING = 2.5
NC = 8
P = 128
NT = T // P            # 8 token tiles
ND = D // P            # 16 d chunks
NI = I // P            # 11 i tiles
SI = SHARED_I // NC    # 352 per-core shard of the shared intermediate
SIP = 384              # padded to 3 tiles of 128
NSI = SIP // P
BIG = 60000.0          # offset for non-selected rows; > bounds -> skipped
COUNT_GUARD = 4

_program_cache = {}


def _routing_counts(x, gate_w, gate_b):
    logits = x.astype(np.float32) @ gate_w.T.astype(np.float32) + gate_b
    scores = 1.0 / (1.0 + np.exp(-logits))
    s4c = scores + gate_b
    grp = s4c.reshape(T, G, E // G)
    m1 = grp.max(-1)
    m2 = np.where(grp == m1[:, :, None], -np.inf, grp).max(-1)
    gs = m1 + m2
    th = np.sort(gs, 1)[:, G - TOPK_GROUP]
    smask = np.repeat(gs >= th[:, None], E // G, axis=1)
    tmp = np.where(smask, s4c, 0.0)
    et = np.sort(tmp, 1)[:, E - TOP_K]
    return (tmp >= et[:, None]).sum(0)


def _assignment(counts):
    blocks = np.maximum(1, np.ceil((counts + COUNT_GUARD) / P)).astype(int)
    order = sorted(range(E), key=lambda e: (-blocks[e], -counts[e]))
    prof = tuple(int(blocks[order[8 * k]]) for k in range(4))
    assign = np.zeros((NC, 4), dtype=int)
    for k in range(4):
        for c in range(NC):
            assign[c, k] = order[8 * k + c]
    return assign, prof


def build_program(prof):
    nc = bacc.Bacc("TRN2", num_devices=NC)
    CK = [P * b for b in prof]
    CKMAX = max(CK)

    xTf = nc.dram_tensor("xTf", [ND, P, T], F32, kind="ExternalInput")
    xbf = nc.dram_tensor("xbf", [T + 1, D], BF16, kind="ExternalInput")
    gwT = nc.dram_tensor("gwT", [ND, P, E], F32, kind="ExternalInput")
    gbias = nc.dram_tensor("gbias", [1, E], F32, kind="ExternalInput")
    ones1 = nc.dram_tensor("ones1", [1, P], F32, kind="ExternalInput")
    ut128 = nc.dram_tensor("ut128", [P, P], F32, kind="ExternalInput")
    wcum = nc.dram_tensor("wcum", [P, NT * NT], F32, kind="ExternalInput")
    ejall = nc.dram_tensor("ejall", [NT, NT * P], F32, kind="ExternalInput")
    onehot = nc.dram_tensor("onehot", [P, 4, E], F32, kind="ExternalInput")
    iota128 = nc.dram_tensor("iota128", [P, 1], F32, kind="ExternalInput")
    identbf = nc.dram_tensor("identbf", [P, P], BF16, kind="ExternalInput")
    initmeta = nc.dram_tensor("initmeta", [CKMAX, 2], F32, kind="ExternalInput")
    wgu = [nc.dram_tensor(f"wgu{k}", [NI, P, ND, P], BF16, kind="ExternalInput")
           for k in range(4)]
    wuu = [nc.dram_tensor(f"wuu{k}", [NI, P, ND, P], BF16, kind="ExternalInput")
           for k in range(4)]
    wdd = [nc.dram_tensor(f"wdd{k}", [4, NI, P, 512], BF16, kind="ExternalInput")
           for k in range(4)]
    swg = nc.dram_tensor("swg", [NSI, P, ND, P], BF16, kind="ExternalInput")
    swu = nc.dram_tensor("swu", [NSI, P, ND, P], BF16, kind="ExternalInput")
    swd = nc.dram_tensor("swd", [4, NSI, P, 512], BF16, kind="ExternalInput")

    ysh = nc.dram_tensor("ysh", [P, D], F32, kind="ExternalOutput")

    AF = mybir.ActivationFunctionType
    ALU = mybir.AluOpType

    with TileContext(nc) as tc:
        with tc.tile_pool(name="dram", bufs=1, space="DRAM") as dpool, \
             tc.tile_pool(name="const", bufs=1) as cpool, \
             tc.tile_pool(name="small", bufs=4) as spool, \
             tc.tile_pool(name="meta", bufs=10) as mpool:

            partial = dpool.tile([T + 1, D], F32, tag="partial")
            rsout = dpool.tile([P, D], F32, tag="rsout")
            glmeta = [dpool.tile([CK[k], 2], F32, tag=f"glmeta{k}", name=f"glmeta{k}")
                      for k in range(4)]

            xTbf = cpool.tile([P, ND, T], BF16, tag="xTbf")
            ut_sb = cpool.tile([P, P], F32, tag="ut")
            wcum_sb = cpool.tile([P, NT * NT], F32, tag="wcum")
            ej_sb = cpool.tile([NT, NT * P], F32, tag="ej")
            oh_sb = cpool.tile([P, 4, E], F32, tag="oh")
            ones1_sb = cpool.tile([1, P], F32, tag="ones1")
            iota_sb = cpool.tile([P, 1], F32, tag="iota")
            gb_sb = cpool.tile([1, E], F32, tag="gb")
            idbf_sb = cpool.tile([P, P], BF16, tag="idbf")
            bias_bc = cpool.tile([P, E], F32, tag="biasbc")
            routew = cpool.tile([P, NT, E], F32, tag="routew")
            poff_all = cpool.tile([P, NT, E], F32, tag="poffall")
            base8 = cpool.tile([NT, E], F32, tag="base8")
            a_sh = cpool.tile([P, NSI, T], BF16, tag="ash")
            zrow = cpool.tile([1, D], F32, tag="zrow")

            nc.sync.dma_start(out=ut_sb[:], in_=ut128[:])
            nc.sync.dma_start(out=wcum_sb[:], in_=wcum[:])
            nc.sync.dma_start(out=ej_sb[:], in_=ejall[:])
            nc.sync.dma_start(out=oh_sb[:], in_=onehot[:])
            nc.sync.dma_start(out=ones1_sb[:], in_=ones1[:])
            nc.sync.dma_start(out=iota_sb[:], in_=iota128[:])
            nc.sync.dma_start(out=gb_sb[:], in_=gbias[:])
            nc.sync.dma_start(out=idbf_sb[:], in_=identbf[:])
            nc.vector.memset(zrow[:], 0.0)
            nc.sync.dma_start(out=partial[T:T + 1, :], in_=zrow[:])
            for k in range(4):
                nc.sync.dma_start(out=glmeta[k][:], in_=initmeta[:CK[k], :])

            # ================= gate (f32) =================
            with tc.tile_pool(name="pgate", bufs=8, space="PSUM") as pg, \
                 tc.tile_pool(name="xtfs", bufs=3) as xpool:
                psb = pg.tile([P, E], F32, tag="psg")
                nc.tensor.matmul(psb[:], ones1_sb[:], gb_sb[:],
                                 start=True, stop=True)
                nc.vector.tensor_copy(out=bias_bc[:], in_=psb[:])
                psg = [pg.tile([P, E], F32, tag="psg", name=f"psg{j}")
                       for j in range(NT)]
                for j in range(NT):
                    nc.tensor.matmul(psg[j][:], ones1_sb[:], gb_sb[:],
                                     start=True, stop=False)
                for dc in range(ND):
                    xf = xpool.tile([P, T], F32, tag="xtf")
                    nc.sync.dma_start(out=xf[:], in_=xTf[dc])
                    gw = xpool.tile([P, E], F32, tag="gw")
                    nc.sync.dma_start(out=gw[:], in_=gwT[dc])
                    nc.vector.tensor_copy(out=xTbf[:, dc, :], in_=xf[:])
                    for j in range(NT):
                        nc.tensor.matmul(psg[j][:], xf[:, j * P:(j + 1) * P],
                                         gw[:], start=False, stop=(dc == ND - 1))

                # ============ routing per token tile ============
                for j in range(NT):
                    scores = spool.tile([P, E], F32, tag="scores")
                    nc.scalar.activation(scores[:], psg[j][:], AF.Sigmoid)
                    s4c = spool.tile([P, E], F32, tag="s4c")
                    nc.vector.tensor_add(s4c[:], scores[:], bias_bc[:])
                    s3 = s4c[:].rearrange("p (g r) -> p g r", r=E // G)
                    m1 = spool.tile([P, G, 1], F32, tag="m1")
                    nc.vector.reduce_max(out=m1[:], in_=s3,
                                         axis=mybir.AxisListType.X)
                    msk = spool.tile([P, E], F32, tag="msk")
                    nc.vector.tensor_tensor(
                        out=msk[:].rearrange("p (g r) -> p g r", r=E // G),
                        in0=s3, in1=m1[:].to_broadcast([P, G, E // G]),
                        op=ALU.is_equal)
                    nc.vector.tensor_scalar(out=msk[:], in0=msk[:], scalar1=1e9,
                                            scalar2=None, op0=ALU.mult)
                    nc.vector.tensor_sub(msk[:], s4c[:], msk[:])
                    m2 = spool.tile([P, G, 1], F32, tag="m2")
                    nc.vector.reduce_max(
                        out=m2[:],
                        in_=msk[:].rearrange("p (g r) -> p g r", r=E // G),
                        axis=mybir.AxisListType.X)
                    gsc = spool.tile([P, G], F32, tag="gsc")
                    nc.vector.tensor_tensor(out=gsc[:], in0=m1[:, :, 0],
                                            in1=m2[:, :, 0], op=ALU.add)
                    nc.vector.tensor_scalar(out=gsc[:], in0=gsc[:], scalar1=4.0,
                                            scalar2=None, op0=ALU.add)
                    mx8 = spool.tile([P, 8], F32, tag="mx8")
                    nc.vector.max(out=mx8[:], in_=gsc[:])
                    nc.vector.memset(mx8[:, TOPK_GROUP:], 0.0)
                    gz = spool.tile([P, G], F32, tag="gz")
                    nc.vector.match_replace(out=gz[:], in_to_replace=mx8[:],
                                            in_values=gsc[:], imm_value=0.0)
                    gmask = spool.tile([P, G], F32, tag="gmask")
                    nc.vector.tensor_sub(gmask[:], gsc[:], gz[:])
                    nc.vector.tensor_scalar(out=gmask[:], in0=gmask[:],
                                            scalar1=0.0, scalar2=None,
                                            op0=ALU.is_gt)
                    stmp = spool.tile([P, E], F32, tag="stmp")
                    nc.vector.tensor_scalar(out=stmp[:], in0=s4c[:], scalar1=4.0,
                                            scalar2=None, op0=ALU.add)
                    nc.vector.tensor_tensor(
                        out=stmp[:].rearrange("p (g r) -> p g r", r=E // G),
                        in0=stmp[:].rearrange("p (g r) -> p g r", r=E // G),
                        in1=gmask[:, :, None].to_broadcast([P, G, E // G]),
                        op=ALU.mult)
                    ex8 = spool.tile([P, 8], F32, tag="ex8")
                    nc.vector.max(out=ex8[:], in_=stmp[:])
                    nc.vector.memset(ex8[:, TOP_K:], 0.0)
                    ez = spool.tile([P, E], F32, tag="ez")
                    nc.vector.match_replace(out=ez[:], in_to_replace=ex8[:],
                                            in_values=stmp[:], imm_value=0.0)
                    emask = spool.tile([P, E], F32, tag="emask")
                    nc.vector.tensor_sub(emask[:], stmp[:], ez[:])
                    nc.vector.tensor_scalar(out=emask[:], in0=emask[:],
                                            scalar1=0.0, scalar2=None,
                                            op0=ALU.is_gt)
                    tw = spool.tile([P, E], F32, tag="tw")
                    nc.vector.tensor_mul(tw[:], scores[:], emask[:])
                    rsum = spool.tile([P, 1], F32, tag="rsum")
                    nc.vector.reduce_sum(out=rsum[:], in_=tw[:],
                                         axis=mybir.AxisListType.X)
                    nc.vector.reciprocal(out=rsum[:], in_=rsum[:])
                    nc.vector.tensor_scalar(out=rsum[:], in0=rsum[:],
                                            scalar1=ROUTED_SCALING,
                                            scalar2=None, op0=ALU.mult)
                    nc.vector.tensor_scalar_mul(out=routew[:, j, :], in0=tw[:],
                                                scalar1=rsum[:])
                    nc.vector.tensor_copy(out=poff_all[:, j, :], in_=emask[:])

            # ============ slot positions (cumsum via matmuls) ============
            with tc.tile_pool(name="ppos", bufs=3, space="PSUM") as pp:
                pb8 = pp.tile([NT, E], F32, tag="pbase")
                for jp in range(NT):
                    nc.tensor.matmul(pb8[:], wcum_sb[:, jp * NT:(jp + 1) * NT],
                                     poff_all[:, jp, :],
                                     start=(jp == 0), stop=(jp == NT - 1))
                nc.vector.tensor_copy(out=base8[:], in_=pb8[:])
                for j in range(NT):
                    ppos = pp.tile([P, E], F32, tag="ppos")
                    nc.tensor.matmul(ppos[:], ut_sb[:], poff_all[:, j, :],
                                     start=True, stop=False)
                    nc.tensor.matmul(ppos[:], ej_sb[:, j * P:(j + 1) * P],
                                     base8[:], start=False, stop=True)
                    t1 = spool.tile([P, E], F32, tag="t1")
                    nc.vector.tensor_scalar(out=t1[:], in0=poff_all[:, j, :],
                                            scalar1=-BIG, scalar2=BIG,
                                            op0=ALU.mult, op1=ALU.add)
                    p2 = spool.tile([P, E], F32, tag="p2")
                    nc.vector.tensor_scalar(out=p2[:], in0=ppos[:], scalar1=1.0,
                                            scalar2=None, op0=ALU.subtract)
                    nc.vector.tensor_mul(p2[:], p2[:], poff_all[:, j, :])
                    nc.vector.tensor_add(poff_all[:, j, :], p2[:], t1[:])

            # ============ dispatch: inversion scatters ============
            for k in range(4):
                for j in range(NT):
                    sel = mpool.tile([P, E], F32, tag="sel")
                    nc.vector.tensor_mul(sel[:], poff_all[:, j, :],
                                         oh_sb[:, k, :])
                    offk = mpool.tile([P, 1], F32, tag="offk")
                    nc.vector.reduce_sum(out=offk[:], in_=sel[:],
                                         axis=mybir.AxisListType.X)
                    offi = mpool.tile([P, 1], I32, tag="offi")
                    nc.vector.tensor_copy(out=offi[:], in_=offk[:])
                    meta = mpool.tile([P, 2], F32, tag="meta")
                    nc.vector.tensor_scalar(out=meta[:, 0:1], in0=iota_sb[:],
                                            scalar1=float(j * P), scalar2=None,
                                            op0=ALU.add)
                    wsel = mpool.tile([P, E], F32, tag="wsel")
                    nc.vector.tensor_mul(wsel[:], routew[:, j, :],
                                         oh_sb[:, k, :])
                    nc.vector.reduce_sum(out=meta[:, 1:2], in_=wsel[:],
                                         axis=mybir.AxisListType.X)
                    nc.gpsimd.indirect_dma_start(
                        out=glmeta[k][:],
                        out_offset=bass.IndirectOffsetOnAxis(ap=offi[:, :1],
                                                             axis=0),
                        in_=meta[:],
                        in_offset=None,
                        bounds_check=CK[k] - 1,
                        oob_is_err=False,
                    )

            # ====== shared expert + routed experts ======
            with tc.tile_pool(name="pt", bufs=2, space="PSUM") as pt, \
                 tc.tile_pool(name="ph", bufs=3, space="PSUM") as ph, \
                 tc.tile_pool(name="py", bufs=3, space="PSUM") as py, \
                 tc.tile_pool(name="wpool", bufs=2) as wpool, \
                 tc.tile_pool(name="wdpool", bufs=6) as wdpool, \
                 tc.tile_pool(name="apool", bufs=2) as apool, \
                 tc.tile_pool(name="gpool", bufs=2) as gpool, \
                 tc.tile_pool(name="ppool", bufs=4) as ppool:

                # shared up/gate
                for th in range(2):
                    tsl = slice(th * 512, (th + 1) * 512)
                    for it in range(NSI):
                        wg_sb = wpool.tile([P, ND, P], BF16, tag="wgu")
                        nc.sync.dma_start(out=wg_sb[:], in_=swg[it])
                        wu_sb = wpool.tile([P, ND, P], BF16, tag="wuu")
                        nc.sync.dma_start(out=wu_sb[:], in_=swu[it])
                        h1 = ph.tile([P, 512], F32, tag="ph")
                        h2 = ph.tile([P, 512], F32, tag="ph")
                        for dc in range(ND):
                            nc.tensor.matmul(h1[:], wg_sb[:, dc, :],
                                             xTbf[:, dc, tsl],
                                             start=(dc == 0),
                                             stop=(dc == ND - 1))
                        for dc in range(ND):
                            nc.tensor.matmul(h2[:], wu_sb[:, dc, :],
                                             xTbf[:, dc, tsl],
                                             start=(dc == 0),
                                             stop=(dc == ND - 1))
                        sl = spool.tile([P, 512], F32, tag="silu")
                        nc.scalar.activation(sl[:], h1[:], AF.Silu)
                        nc.vector.tensor_tensor(out=a_sh[:, it, tsl],
                                                in0=sl[:], in1=h2[:],
                                                op=ALU.mult)
                # shared down -> init partial
                for db in range(4):
                    dsl = slice(db * 512, (db + 1) * 512)
                    wd_l = []
                    for ic in range(NSI):
                        w = wdpool.tile([P, 512], BF16, tag="wd")
                        nc.sync.dma_start(out=w[:], in_=swd[db, ic])
                        wd_l.append(w)
                    for ts in range(NT):
                        pys = py.tile([P, 512], F32, tag="py")
                        for ic in range(NSI):
                            nc.tensor.matmul(pys[:],
                                             a_sh[:, ic, ts * P:(ts + 1) * P],
                                             wd_l[ic][:],
                                             start=(ic == 0),
                                             stop=(ic == NSI - 1))
                        outt = spool.tile([P, 512], F32, tag="sout")
                        nc.vector.tensor_copy(out=outt[:], in_=pys[:])
                        nc.sync.dma_start(out=partial[ts * P:(ts + 1) * P, dsl],
                                          in_=outt[:])

                # routed experts
                for k in range(4):
                    ck = CK[k]
                    nblk = prof[k]
                    toki, wcol = [], []
                    for b in range(nblk):
                        ms = mpool.tile([P, 2], F32, tag="metald")
                        nc.sync.dma_start(out=ms[:],
                                          in_=glmeta[k][b * P:(b + 1) * P, :])
                        ti = mpool.tile([P, 1], I32, tag="toki")
                        nc.vector.tensor_copy(out=ti[:], in_=ms[:, 0:1])
                        toki.append(ti)
                        wcol.append(ms)
                    xgT = gpool.tile([P, ND, ck], BF16, tag="xgT")
                    for b in range(nblk):
                        xg = gpool.tile([P, D], BF16, tag="xg")
                        nc.gpsimd.indirect_dma_start(
                            out=xg[:], out_offset=None,
                            in_=xbf[:],
                            in_offset=bass.IndirectOffsetOnAxis(
                                ap=toki[b][:, :1], axis=0),
                        )
                        for dc in range(ND):
                            ptt = pt.tile([P, P], BF16, tag="pt")
                            nc.tensor.transpose(ptt[:],
                                                xg[:, dc * P:(dc + 1) * P],
                                                idbf_sb[:])
                            nc.vector.tensor_copy(
                                out=xgT[:, dc, b * P:(b + 1) * P], in_=ptt[:])
                    a_e = apool.tile([P, NI, ck], BF16, tag="ae")
                    for it in range(NI):
                        wg_sb = wpool.tile([P, ND, P], BF16, tag="wgu")
                        nc.sync.dma_start(out=wg_sb[:], in_=wgu[k][it])
                        wu_sb = wpool.tile([P, ND, P], BF16, tag="wuu")
                        nc.sync.dma_start(out=wu_sb[:], in_=wuu[k][it])
                        h1 = ph.tile([P, 512], F32, tag="ph")
                        h2 = ph.tile([P, 512], F32, tag="ph")
                        for dc in range(ND):
                            nc.tensor.matmul(h1[:, :ck], wg_sb[:, dc, :],
                                             xgT[:, dc, :],
                                             start=(dc == 0),
                                             stop=(dc == ND - 1))
                        for dc in range(ND):
                            nc.tensor.matmul(h2[:, :ck], wu_sb[:, dc, :],
                                             xgT[:, dc, :],
                                             start=(dc == 0),
                                             stop=(dc == ND - 1))
                        sl = spool.tile([P, 512], F32, tag="silu")
                        nc.scalar.activation(sl[:, :ck], h1[:, :ck], AF.Silu)
                        nc.vector.tensor_tensor(out=a_e[:, it, :],
                                                in0=sl[:, :ck], in1=h2[:, :ck],
                                                op=ALU.mult)
                    # gather current partial rows for this expert's tokens
                    padds = []
                    for b in range(nblk):
                        padd = ppool.tile([P, D], F32, tag="padd")
                        nc.gpsimd.indirect_dma_start(
                            out=padd[:], out_offset=None,
                            in_=partial[:],
                            in_offset=bass.IndirectOffsetOnAxis(
                                ap=toki[b][:, :1], axis=0),
                        )
                        padds.append(padd)
                    # down projection, weighting, accumulate into padd
                    for db in range(4):
                        dsl = slice(db * 512, (db + 1) * 512)
                        pys = [py.tile([P, 512], F32, tag="py", name=f"pys{b}")
                               for b in range(nblk)]
                        for ic in range(NI):
                            wd_sb = wdpool.tile([P, 512], BF16, tag="wd")
                            nc.sync.dma_start(out=wd_sb[:], in_=wdd[k][db, ic])
                            for b in range(nblk):
                                nc.tensor.matmul(pys[b][:],
                                                 a_e[:, ic, b * P:(b + 1) * P],
                                                 wd_sb[:],
                                                 start=(ic == 0),
                                                 stop=(ic == NI - 1))
                        for b in range(nblk):
                            nc.vector.tensor_scalar_mul(
                                out=pys[b][:], in0=pys[b][:],
                                scalar1=wcol[b][:, 1:2])
                            nc.vector.tensor_add(padds[b][:, dsl],
                                                 padds[b][:, dsl], pys[b][:])
                    for b in range(nblk):
                        nc.gpsimd.indirect_dma_start(
                            out=partial[:],
                            out_offset=bass.IndirectOffsetOnAxis(
                                ap=toki[b][:, :1], axis=0),
                            in_=padds[b][:],
                            in_offset=None,
                        )

            # ================= reduce-scatter + output =================
            nc.gpsimd.collective_compute(
                "ReduceScatter",
                mybir.AluOpType.add,
                ins=[partial[0:T, :]],
                outs=[rsout[:]],
                replica_groups=[list(range(NC))],
            )
            with tc.tile_pool(name="opool", bufs=1) as opool:
                yo = opool.tile([P, D], F32, tag="yo")
                nc.sync.dma_start(out=yo[:], in_=rsout[:])
                nc.sync.dma_start(out=ysh[:], in_=yo[:])

    nc.finalize()
    return nc


def _bf16(a):
    return np.ascontiguousarray(a.astype(ml_dtypes.bfloat16))


def _prep_inputs(x, gate_w, gate_b, w_gate, w_up, w_down, sw_gate, sw_up,
                 sw_down, assign, prof):
    CKMAX = P * max(prof)
    xT = np.ascontiguousarray(x.T.astype(np.float32))
    com = {
        "xTf": xT.reshape(ND, P, T),
        "xbf": _bf16(np.concatenate([x, np.zeros((1, D), np.float32)], axis=0)),
        "gwT": np.ascontiguousarray(gate_w.T.astype(np.float32)).reshape(ND, P, E),
        "gbias": gate_b.reshape(1, E).astype(np.float32),
        "ones1": np.ones((1, P), np.float32),
        "ut128": np.triu(np.ones((P, P), np.float32)),  # ut[t',t]=1 if t'<=t
        "iota128": np.arange(P, dtype=np.float32).reshape(P, 1),
        "identbf": _bf16(np.eye(P, dtype=np.float32)),
        "initmeta": np.tile(np.array([[float(T), 0.0]], np.float32),
                            (CKMAX, 1)),
    }
    wc = np.zeros((P, NT * NT), np.float32)
    for jp in range(NT):
        for j in range(NT):
            if jp < j:
                wc[:, jp * NT + j] = 1.0
    com["wcum"] = wc
    ej = np.zeros((NT, NT * P), np.float32)
    for j in range(NT):
        ej[j, j * P:(j + 1) * P] = 1.0
    com["ejall"] = ej

    sgp = np.zeros((D, SIP), np.float32)
    sup = np.zeros((D, SIP), np.float32)
    sdp = np.zeros((SIP, D), np.float32)

    in_maps = []
    for c in range(NC):
        m = dict(com)
        oh = np.zeros((P, 4, E), np.float32)
        for k in range(4):
            oh[:, k, assign[c, k]] = 1.0
        m["onehot"] = oh
        for k in range(4):
            e = assign[c, k]
            m[f"wgu{k}"] = _bf16(rearrange(w_gate[e], "(dc p) (it i) -> it p dc i",
                                           p=P, i=P))
            m[f"wuu{k}"] = _bf16(rearrange(w_up[e], "(dc p) (it i) -> it p dc i",
                                           p=P, i=P))
            m[f"wdd{k}"] = _bf16(rearrange(w_down[e], "(ic p) (db d) -> db ic p d",
                                           p=P, d=512))
        sgp[:, :SI] = sw_gate[:, c * SI:(c + 1) * SI]
        sup[:, :SI] = sw_up[:, c * SI:(c + 1) * SI]
        sdp[:SI, :] = sw_down[c * SI:(c + 1) * SI, :]
        m["swg"] = _bf16(rearrange(sgp, "(dc p) (it i) -> it p dc i", p=P, i=P))
        m["swu"] = _bf16(rearrange(sup, "(dc p) (it i) -> it p dc i", p=P, i=P))
        m["swd"] = _bf16(rearrange(sdp, "(ic p) (db d) -> db ic p d", p=P, d=512))
        in_maps.append(m)
    return in_maps


def _run(inputs, trace=False):
    x = np.asarray(inputs["x"], np.float32)
    gate_w = np.asarray(inputs["gate_w"], np.float32)
    gate_b = np.asarray(inputs["gate_b"], np.float32)
    counts = _routing_counts(x, gate_w, gate_b)
    assign, prof = _assignment(counts)
    if prof not in _program_cache:
        _program_cache[prof] = build_program(prof)
    nc = _program_cache[prof]
    in_maps = _prep_inputs(
        x, gate_w, gate_b,
        np.asarray(inputs["w_gate"], np.float32),
        np.asarray(inputs["w_up"], np.float32),
        np.asarray(inputs["w_down"], np.float32),
        np.asarray(inputs["sw_gate"], np.float32),
        np.asarray(inputs["sw_up"], np.float32),
        np.asarray(inputs["sw_down"], np.float32),
        assign, prof)
    res = run_bass_kernel_spmd(nc, in_maps, core_ids=list(range(NC)),
                               trace=trace)
    out = np.concatenate([res.results[c]["ysh"] for c in range(NC)], axis=0)
    return out.astype(np.float32), res


def kernel(**inputs):
    out, _ = _run(inputs, trace=False)
    return out


# revision 5
# speedup vs baseline: 1.0743x; 1.0743x over previous
"""DeepseekV3 MoE (T=1024, D=2048, E=32, grouped top-4 routing, I=1408,
shared expert 2816) on 8 trn2 NeuronCores via Bass/Tile.

Expert-parallel sparse dispatch; see build_program for the device-side
structure. Host computes the gate forward once (numpy) ONLY to size
per-expert capacities and balance the expert->(core,slot) assignment;
all model math runs on device.
"""

import numpy as np
import ml_dtypes
from einops import rearrange

import concourse.bass as bass
import concourse.bacc as bacc
import concourse.mybir as mybir
from concourse.tile import TileContext
from concourse.bass_utils import run_bass_kernel_spmd

F32 = mybir.dt.float32
BF16 = mybir.dt.bfloat16
I32 = mybir.dt.int32

T, D, E, I = 1024, 2048, 32, 1408
G, TOPK_GROUP, TOP_K = 8, 4, 4
SHARED_I = 2816
ROUTED_SCALING = 2.5
NC = 8
P = 128
NT = T // P            # 8 token tiles
ND = D // P            # 16 d chunks
NI = I // P            # 11 i tiles
NIP = 12               # padded (pairs of 2 for w_down loads)
SI = SHARED_I // NC    # 352 per-core shard of the shared intermediate
SIP = 384              # padded to 3 tiles of 128
NSI = SIP // P
NSIP = 4               # padded for w_down pairs
BIG = 60000.0          # offset for non-selected rows; > bounds -> skipped
COUNT_GUARD = 4
PARTIAL_BF16 = True    # partial/combine/reduce-scatter in bf16

PDT = BF16 if PARTIAL_BF16 else F32

_program_cache = {}


def _routing_counts(x, gate_w, gate_b):
    logits = x.astype(np.float32) @ gate_w.T.astype(np.float32) + gate_b
    scores = 1.0 / (1.0 + np.exp(-logits))
    s4c = scores + gate_b
    grp = s4c.reshape(T, G, E // G)
    m1 = grp.max(-1)
    m2 = np.where(grp == m1[:, :, None], -np.inf, grp).max(-1)
    gs = m1 + m2
    th = np.sort(gs, 1)[:, G - TOPK_GROUP]
    smask = np.repeat(gs >= th[:, None], E // G, axis=1)
    tmp = np.where(smask, s4c, 0.0)
    et = np.sort(tmp, 1)[:, E - TOP_K]
    return (tmp >= et[:, None]).sum(0)


def _assignment(counts):
    blocks = np.maximum(1, np.ceil((counts + COUNT_GUARD) / P)).astype(int)
    order = sorted(range(E), key=lambda e: (-blocks[e], -counts[e]))
    prof = tuple(int(blocks[order[8 * k]]) for k in range(4))
    assign = np.zeros((NC, 4), dtype=int)
    for k in range(4):
        for c in range(NC):
            assign[c, k] = order[8 * k + c]
    return assign, prof


def build_program(prof):
    nc = bacc.Bacc("TRN2", num_devices=NC)
    CK = [P * b for b in prof]
    CKMAX = max(CK)

    xTf = nc.dram_tensor("xTf", [ND, P, T], F32, kind="ExternalInput")
    xbf = nc.dram_tensor("xbf", [T + 1, D], BF16, kind="ExternalInput")
    gwT = nc.dram_tensor("gwT", [ND, P, E], F32, kind="ExternalInput")
    gbias = nc.dram_tensor("gbias", [1, E], F32, kind="ExternalInput")
    ones1 = nc.dram_tensor("ones1", [1, P], F32, kind="ExternalInput")
    ut128 = nc.dram_tensor("ut128", [P, P], F32, kind="ExternalInput")
    wcum = nc.dram_tensor("wcum", [P, NT * NT], F32, kind="ExternalInput")
    ejall = nc.dram_tensor("ejall", [NT, NT * P], F32, kind="ExternalInput")
    onehot = nc.dram_tensor("onehot", [P, 4, E], F32, kind="ExternalInput")
    iota128 = nc.dram_tensor("iota128", [P, 1], F32, kind="ExternalInput")
    identbf = nc.dram_tensor("identbf", [P, P], BF16, kind="ExternalInput")
    initmeta = nc.dram_tensor("initmeta", [CKMAX, 2], F32, kind="ExternalInput")
    wgu = [nc.dram_tensor(f"wgu{k}", [NI, P, ND, P], BF16, kind="ExternalInput")
           for k in range(4)]
    wuu = [nc.dram_tensor(f"wuu{k}", [NI, P, ND, P], BF16, kind="ExternalInput")
           for k in range(4)]
    wdd = [nc.dram_tensor(f"wdd{k}", [4, NIP // 2, P, 1024], BF16,
                          kind="ExternalInput") for k in range(4)]
    swg = nc.dram_tensor("swg", [NSI, P, ND, P], BF16, kind="ExternalInput")
    swu = nc.dram_tensor("swu", [NSI, P, ND, P], BF16, kind="ExternalInput")
    swd = nc.dram_tensor("swd", [4, NSIP // 2, P, 1024], BF16,
                         kind="ExternalInput")

    ysh = nc.dram_tensor("ysh", [P, D], F32, kind="ExternalOutput")

    AF = mybir.ActivationFunctionType
    ALU = mybir.AluOpType

    with TileContext(nc) as tc:
        with tc.tile_pool(name="dram", bufs=1, space="DRAM") as dpool, \
             tc.tile_pool(name="const", bufs=1) as cpool, \
             tc.tile_pool(name="small", bufs=4) as spool, \
             tc.tile_pool(name="meta", bufs=10) as mpool:

            partial = dpool.tile([T + 1, D], PDT, tag="partial")
            rsout = dpool.tile([P, D], PDT, tag="rsout")
            glmeta = [dpool.tile([CK[k], 2], F32, tag=f"glmeta{k}",
                                 name=f"glmeta{k}") for k in range(4)]

            xTbf = cpool.tile([P, ND, T], BF16, tag="xTbf")
            ut_sb = cpool.tile([P, P], F32, tag="ut")
            wcum_sb = cpool.tile([P, NT * NT], F32, tag="wcum")
            ej_sb = cpool.tile([NT, NT * P], F32, tag="ej")
            oh_sb = cpool.tile([P, 4, E], F32, tag="oh")
            ones1_sb = cpool.tile([1, P], F32, tag="ones1")
            iota_sb = cpool.tile([P, 1], F32, tag="iota")
            gb_sb = cpool.tile([1, E], F32, tag="gb")
            idbf_sb = cpool.tile([P, P], BF16, tag="idbf")
            bias_bc = cpool.tile([P, E], F32, tag="biasbc")
            routew = cpool.tile([P, NT, E], F32, tag="routew")
            poff_all = cpool.tile([P, NT, E], F32, tag="poffall")
            base8 = cpool.tile([NT, E], F32, tag="base8")
            a_sh = cpool.tile([P, NSIP, T], BF16, tag="ash")
            zrow = cpool.tile([1, D], PDT, tag="zrow")

            nc.sync.dma_start(out=ut_sb[:], in_=ut128[:])
            nc.sync.dma_start(out=wcum_sb[:], in_=wcum[:])
            nc.sync.dma_start(out=ej_sb[:], in_=ejall[:])
            nc.sync.dma_start(out=oh_sb[:], in_=onehot[:])
            nc.sync.dma_start(out=ones1_sb[:], in_=ones1[:])
            nc.sync.dma_start(out=iota_sb[:], in_=iota128[:])
            nc.sync.dma_start(out=gb_sb[:], in_=gbias[:])
            nc.sync.dma_start(out=idbf_sb[:], in_=identbf[:])
            nc.vector.memset(zrow[:], 0.0)
            nc.sync.dma_start(out=partial[T:T + 1, :], in_=zrow[:])
            nc.vector.memset(a_sh[:, NSIP - 1, :], 0.0)
            for k in range(4):
                nc.sync.dma_start(out=glmeta[k][:], in_=initmeta[:CK[k], :])

            # ================= gate (f32) =================
            with tc.tile_pool(name="pgate", bufs=8, space="PSUM") as pg, \
                 tc.tile_pool(name="xtfs", bufs=3) as xpool:
                psb = pg.tile([P, E], F32, tag="psg")
                nc.tensor.matmul(psb[:], ones1_sb[:], gb_sb[:],
                                 start=True, stop=True)
                nc.vector.tensor_copy(out=bias_bc[:], in_=psb[:])
                psg = [pg.tile([P, E], F32, tag="psg", name=f"psg{j}")
                       for j in range(NT)]
                for j in range(NT):
                    nc.tensor.matmul(psg[j][:], ones1_sb[:], gb_sb[:],
                                     start=True, stop=False)
                for dc in range(ND):
                    xf = xpool.tile([P, T], F32, tag="xtf")
                    nc.sync.dma_start(out=xf[:], in_=xTf[dc])
                    gw = xpool.tile([P, E], F32, tag="gw")
                    nc.sync.dma_start(out=gw[:], in_=gwT[dc])
                    nc.scalar.activation(xTbf[:, dc, :], xf[:], AF.Copy)
                    for j in range(NT):
                        nc.tensor.matmul(psg[j][:], xf[:, j * P:(j + 1) * P],
                                         gw[:], start=False, stop=(dc == ND - 1))

                # ============ routing per token tile ============
                for j in range(NT):
                    scores = spool.tile([P, E], F32, tag="scores")
                    nc.scalar.activation(scores[:], psg[j][:], AF.Sigmoid)
                    s4c = spool.tile([P, E], F32, tag="s4c")
                    nc.vector.tensor_add(s4c[:], scores[:], bias_bc[:])
                    s3 = s4c[:].rearrange("p (g r) -> p g r", r=E // G)
                    m1 = spool.tile([P, G, 1], F32, tag="m1")
                    nc.vector.reduce_max(out=m1[:], in_=s3,
                                         axis=mybir.AxisListType.X)
                    msk = spool.tile([P, E], F32, tag="msk")
                    nc.vector.tensor_tensor(
                        out=msk[:].rearrange("p (g r) -> p g r", r=E // G),
                        in0=s3, in1=m1[:].to_broadcast([P, G, E // G]),
                        op=ALU.is_equal)
                    nc.vector.tensor_scalar(out=msk[:], in0=msk[:], scalar1=1e9,
                                            scalar2=None, op0=ALU.mult)
                    nc.vector.tensor_sub(msk[:], s4c[:], msk[:])
                    m2 = spool.tile([P, G, 1], F32, tag="m2")
                    nc.vector.reduce_max(
                        out=m2[:],
                        in_=msk[:].rearrange("p (g r) -> p g r", r=E // G),
                        axis=mybir.AxisListType.X)
                    gsc = spool.tile([P, G], F32, tag="gsc")
                    nc.vector.tensor_tensor(out=gsc[:], in0=m1[:, :, 0],
                                            in1=m2[:, :, 0], op=ALU.add)
                    nc.vector.tensor_scalar(out=gsc[:], in0=gsc[:], scalar1=4.0,
                                            scalar2=None, op0=ALU.add)
                    mx8 = spool.tile([P, 8], F32, tag="mx8")
                    nc.vector.max(out=mx8[:], in_=gsc[:])
                    nc.vector.memset(mx8[:, TOPK_GROUP:], 0.0)
                    gz = spool.tile([P, G], F32, tag="gz")
                    nc.vector.match_replace(out=gz[:], in_to_replace=mx8[:],
                                            in_values=gsc[:], imm_value=0.0)
                    gmask = spool.tile([P, G], F32, tag="gmask")
                    nc.vector.tensor_sub(gmask[:], gsc[:], gz[:])
                    nc.vector.tensor_scalar(out=gmask[:], in0=gmask[:],
                                            scalar1=0.0, scalar2=None,
                                            op0=ALU.is_gt)
                    stmp = spool.tile([P, E], F32, tag="stmp")
                    nc.vector.tensor_scalar(out=stmp[:], in0=s4c[:], scalar1=4.0,
                                            scalar2=None, op0=ALU.add)
                    nc.vector.tensor_tensor(
                        out=stmp[:].rearrange("p (g r) -> p g r", r=E // G),
                        in0=stmp[:].rearrange("p (g r) -> p g r", r=E // G),
                        in1=gmask[:, :, None].to_broadcast([P, G, E // G]),
                        op=ALU.mult)
                    ex8 = spool.tile([P, 8], F32, tag="ex8")
                    nc.vector.max(out=ex8[:], in_=stmp[:])
                    nc.vector.memset(ex8[:, TOP_K:], 0.0)
                    ez = spool.tile([P, E], F32, tag="ez")
                    nc.vector.match_replace(out=ez[:], in_to_replace=ex8[:],
                                            in_values=stmp[:], imm_value=0.0)
                    emask = spool.tile([P, E], F32, tag="emask")
                    nc.vector.tensor_sub(emask[:], stmp[:], ez[:])
                    nc.vector.tensor_scalar(out=emask[:], in0=emask[:],
                                            scalar1=0.0, scalar2=None,
                                            op0=ALU.is_gt)
                    tw = spool.tile([P, E], F32, tag="tw")
                    nc.vector.tensor_mul(tw[:], scores[:], emask[:])
                    rsum = spool.tile([P, 1], F32, tag="rsum")
                    nc.vector.reduce_sum(out=rsum[:], in_=tw[:],
                                         axis=mybir.AxisListType.X)
                    nc.vector.reciprocal(out=rsum[:], in_=rsum[:])
                    nc.vector.tensor_scalar(out=rsum[:], in0=rsum[:],
                                            scalar1=ROUTED_SCALING,
                                            scalar2=None, op0=ALU.mult)
                    nc.vector.tensor_scalar_mul(out=routew[:, j, :], in0=tw[:],
                                                scalar1=rsum[:])
                    nc.vector.tensor_copy(out=poff_all[:, j, :], in_=emask[:])

            # ============ slot positions (cumsum via matmuls) ============
            with tc.tile_pool(name="ppos", bufs=3, space="PSUM") as pp:
                pb8 = pp.tile([NT, E], F32, tag="pbase")
                for jp in range(NT):
                    nc.tensor.matmul(pb8[:], wcum_sb[:, jp * NT:(jp + 1) * NT],
                                     poff_all[:, jp, :],
                                     start=(jp == 0), stop=(jp == NT - 1))
                nc.vector.tensor_copy(out=base8[:], in_=pb8[:])
                for j in range(NT):
                    ppos = pp.tile([P, E], F32, tag="ppos")
                    nc.tensor.matmul(ppos[:], ut_sb[:], poff_all[:, j, :],
                                     start=True, stop=False)
                    nc.tensor.matmul(ppos[:], ej_sb[:, j * P:(j + 1) * P],
                                     base8[:], start=False, stop=True)
                    t1 = spool.tile([P, E], F32, tag="t1")
                    nc.vector.tensor_scalar(out=t1[:], in0=poff_all[:, j, :],
                                            scalar1=-BIG, scalar2=BIG,
                                            op0=ALU.mult, op1=ALU.add)
                    p2 = spool.tile([P, E], F32, tag="p2")
                    nc.vector.tensor_scalar(out=p2[:], in0=ppos[:], scalar1=1.0,
                                            scalar2=None, op0=ALU.subtract)
                    nc.vector.tensor_mul(p2[:], p2[:], poff_all[:, j, :])
                    nc.vector.tensor_add(poff_all[:, j, :], p2[:], t1[:])

            # ============ dispatch: inversion scatters ============
            for k in range(4):
                for j in range(NT):
                    sel = mpool.tile([P, E], F32, tag="sel")
                    nc.vector.tensor_mul(sel[:], poff_all[:, j, :],
                                         oh_sb[:, k, :])
                    offk = mpool.tile([P, 1], F32, tag="offk")
                    nc.vector.reduce_sum(out=offk[:], in_=sel[:],
                                         axis=mybir.AxisListType.X)
                    offi = mpool.tile([P, 1], I32, tag="offi")
                    nc.vector.tensor_copy(out=offi[:], in_=offk[:])
                    meta = mpool.tile([P, 2], F32, tag="meta")
                    nc.vector.tensor_scalar(out=meta[:, 0:1], in0=iota_sb[:],
                                            scalar1=float(j * P), scalar2=None,
                                            op0=ALU.add)
                    wsel = mpool.tile([P, E], F32, tag="wsel")
                    nc.vector.tensor_mul(wsel[:], routew[:, j, :],
                                         oh_sb[:, k, :])
                    nc.vector.reduce_sum(out=meta[:, 1:2], in_=wsel[:],
                                         axis=mybir.AxisListType.X)
                    nc.gpsimd.indirect_dma_start(
                        out=glmeta[k][:],
                        out_offset=bass.IndirectOffsetOnAxis(ap=offi[:, :1],
                                                             axis=0),
                        in_=meta[:],
                        in_offset=None,
                        bounds_check=CK[k] - 1,
                        oob_is_err=False,
                    )

            # ====== shared expert + routed experts ======
            with tc.tile_pool(name="pt", bufs=2, space="PSUM") as pt, \
                 tc.tile_pool(name="ph", bufs=3, space="PSUM") as ph, \
                 tc.tile_pool(name="py", bufs=3, space="PSUM") as py, \
                 tc.tile_pool(name="wpool", bufs=3) as wpool, \
                 tc.tile_pool(name="wdpool", bufs=3) as wdpool, \
                 tc.tile_pool(name="apool", bufs=2) as apool, \
                 tc.tile_pool(name="gpool", bufs=2) as gpool, \
                 tc.tile_pool(name="ypool", bufs=4) as ypool:

                # shared up/gate
                for th in range(2):
                    tsl = slice(th * 512, (th + 1) * 512)
                    for it in range(NSI):
                        wg_sb = wpool.tile([P, ND, P], BF16, tag="wgu")
                        nc.sync.dma_start(out=wg_sb[:], in_=swg[it])
                        wu_sb = wpool.tile([P, ND, P], BF16, tag="wuu")
                        nc.sync.dma_start(out=wu_sb[:], in_=swu[it])
                        h1 = ph.tile([P, 512], F32, tag="ph")
                        h2 = ph.tile([P, 512], F32, tag="ph")
                        for dc in range(ND):
                            nc.tensor.matmul(h1[:], wg_sb[:, dc, :],
                                             xTbf[:, dc, tsl],
                                             start=(dc == 0),
                                             stop=(dc == ND - 1))
                        for dc in range(ND):
                            nc.tensor.matmul(h2[:], wu_sb[:, dc, :],
                                             xTbf[:, dc, tsl],
                                             start=(dc == 0),
                                             stop=(dc == ND - 1))
                        sl = spool.tile([P, 512], F32, tag="silu")
                        nc.scalar.activation(sl[:], h1[:], AF.Silu)
                        nc.vector.tensor_tensor(out=a_sh[:, it, tsl],
                                                in0=sl[:], in1=h2[:],
                                                op=ALU.mult)
                # shared down -> init partial
                for db in range(4):
                    dsl = slice(db * 512, (db + 1) * 512)
                    wd_l = []
                    for icp in range(NSIP // 2):
                        w = wdpool.tile([P, 1024], BF16, tag="wd")
                        nc.sync.dma_start(out=w[:], in_=swd[db, icp])
                        wd_l.append(w)
                    for ts in range(NT):
                        pys = py.tile([P, 512], F32, tag="py")
                        for ic in range(NSIP):
                            nc.tensor.matmul(
                                pys[:],
                                a_sh[:, ic, ts * P:(ts + 1) * P],
                                wd_l[ic // 2][:, (ic % 2) * 512:(ic % 2) * 512 + 512],
                                start=(ic == 0), stop=(ic == NSIP - 1))
                        outt = spool.tile([P, 512], PDT, tag="sout")
                        nc.vector.tensor_copy(out=outt[:], in_=pys[:])
                        nc.sync.dma_start(out=partial[ts * P:(ts + 1) * P, dsl],
                                          in_=outt[:])

                # routed experts
                for k in range(4):
                    ck = CK[k]
                    nblk = prof[k]
                    toki = []
                    for b in range(nblk):
                        ms = mpool.tile([P, 2], F32, tag="metald")
                        nc.sync.dma_start(out=ms[:],
                                          in_=glmeta[k][b * P:(b + 1) * P, :])
                        ti = mpool.tile([P, 1], I32, tag="toki")
                        nc.vector.tensor_copy(out=ti[:], in_=ms[:, 0:1])
                        toki.append(ti)
                    # broadcast per-slot routing weights: [1, ck] -> [128, ck]
                    wrow = mpool.tile([1, CKMAX], F32, tag="wrow")
                    nc.sync.dma_start(out=wrow[:, :ck], in_=glmeta[k][:, 1:2])
                    wb_ps = ph.tile([P, CKMAX], F32, tag="ph")
                    nc.tensor.matmul(wb_ps[:, :ck], ones1_sb[:], wrow[:, :ck],
                                     start=True, stop=True)
                    wb_sb = mpool.tile([P, CKMAX], F32, tag="wbsb")
                    nc.vector.tensor_copy(out=wb_sb[:, :ck], in_=wb_ps[:, :ck])
                    # gather x rows and transpose to [d, slots]
                    xgT = gpool.tile([P, ND, ck], BF16, tag="xgT")
                    for b in range(nblk):
                        xg = gpool.tile([P, D], BF16, tag="xg")
                        nc.gpsimd.indirect_dma_start(
                            out=xg[:], out_offset=None,
                            in_=xbf[:],
                            in_offset=bass.IndirectOffsetOnAxis(
                                ap=toki[b][:, :1], axis=0),
                        )
                        for dc in range(ND):
                            ptt = pt.tile([P, P], BF16, tag="pt")
                            nc.tensor.transpose(ptt[:],
                                                xg[:, dc * P:(dc + 1) * P],
                                                idbf_sb[:])
                            nc.vector.tensor_copy(
                                out=xgT[:, dc, b * P:(b + 1) * P], in_=ptt[:])
                    # up/gate; a_e pre-scaled by routing weight
                    a_e = apool.tile([P, NIP, ck], BF16, tag="ae")
                    nc.vector.memset(a_e[:, NIP - 1, :], 0.0)
                    for it in range(NI):
                        wg_sb = wpool.tile([P, ND, P], BF16, tag="wgu")
                        nc.sync.dma_start(out=wg_sb[:], in_=wgu[k][it])
                        wu_sb = wpool.tile([P, ND, P], BF16, tag="wuu")
                        nc.sync.dma_start(out=wu_sb[:], in_=wuu[k][it])
                        h1 = ph.tile([P, 512], F32, tag="ph")
                        h2 = ph.tile([P, 512], F32, tag="ph")
                        for dc in range(ND):
                            nc.tensor.matmul(h1[:, :ck], wg_sb[:, dc, :],
                                             xgT[:, dc, :],
                                             start=(dc == 0),
                                             stop=(dc == ND - 1))
                        for dc in range(ND):
                            nc.tensor.matmul(h2[:, :ck], wu_sb[:, dc, :],
                                             xgT[:, dc, :],
                                             start=(dc == 0),
                                             stop=(dc == ND - 1))
                        sl = spool.tile([P, 512], F32, tag="silu")
                        nc.scalar.activation(sl[:, :ck], h1[:, :ck], AF.Silu)
                        nc.vector.tensor_mul(sl[:, :ck], sl[:, :ck], h2[:, :ck])
                        nc.vector.tensor_tensor(out=a_e[:, it, :],
                                                in0=sl[:, :ck],
                                                in1=wb_sb[:, :ck],
                                                op=ALU.mult)
                    # down projection -> weighted y, staged then scatter-added
                    ysts = [ypool.tile([P, D], PDT, tag="yst", name=f"yst{b}")
                            for b in range(nblk)]
                    for db in range(4):
                        dsl = slice(db * 512, (db + 1) * 512)
                        pys = [py.tile([P, 512], F32, tag="py", name=f"pys{b}")
                               for b in range(nblk)]
                        for icp in range(NIP // 2):
                            wd_sb = wdpool.tile([P, 1024], BF16, tag="wd")
                            nc.sync.dma_start(out=wd_sb[:], in_=wdd[k][db, icp])
                            for jj in range(2):
                                ic = icp * 2 + jj
                                for b in range(nblk):
                                    nc.tensor.matmul(
                                        pys[b][:],
                                        a_e[:, ic, b * P:(b + 1) * P],
                                        wd_sb[:, jj * 512:jj * 512 + 512],
                                        start=(ic == 0), stop=(ic == NIP - 1))
                        for b in range(nblk):
                            nc.vector.tensor_copy(out=ysts[b][:, dsl],
                                                  in_=pys[b][:])
                    for b in range(nblk):
                        nc.gpsimd.indirect_dma_start(
                            out=partial[:],
                            out_offset=bass.IndirectOffsetOnAxis(
                                ap=toki[b][:, :1], axis=0),
                            in_=ysts[b][:],
                            in_offset=None,
                            compute_op=ALU.add,
                        )

            # ================= reduce-scatter + output =================
            nc.gpsimd.collective_compute(
                "ReduceScatter",
                mybir.AluOpType.add,
                ins=[partial[0:T, :]],
                outs=[rsout[:]],
                replica_groups=[list(range(NC))],
            )
            with tc.tile_pool(name="opool", bufs=1) as opool:
                yo = opool.tile([P, D], PDT, tag="yo")
                nc.sync.dma_start(out=yo[:], in_=rsout[:])
                if PARTIAL_BF16:
                    yof = opool.tile([P, D], F32, tag="yof")
                    nc.vector.tensor_copy(out=yof[:], in_=yo[:])
                    nc.sync.dma_start(out=ysh[:], in_=yof[:])
                else:
                    nc.sync.dma_start(out=ysh[:], in_=yo[:])

    nc.finalize()
    return nc


def _bf16(a):
    return np.ascontiguousarray(a.astype(ml_dtypes.bfloat16))


def _wd_swizzle(wd, nip):
    """[I', D] -> [4 dblocks, nip/2 ic-pairs, 128, 1024] with 2KB partition
    lines: line (db, icp, p) = [wd[2*icp*128+p, db*512:+512], wd[(2*icp+1)*128+p, ...]]"""
    ipad = nip * P
    wdp = np.zeros((ipad, D), np.float32)
    wdp[:wd.shape[0]] = wd
    # [ic, p, db, d] -> [db, icp, p, (j d)]
    a = wdp.reshape(nip, P, 4, 512)
    a = a.reshape(nip // 2, 2, P, 4, 512)
    out = np.transpose(a, (3, 0, 2, 1, 4)).reshape(4, nip // 2, P, 1024)
    return _bf16(out)


def _prep_inputs(x, gate_w, gate_b, w_gate, w_up, w_down, sw_gate, sw_up,
                 sw_down, assign, prof):
    CKMAX = P * max(prof)
    xT = np.ascontiguousarray(x.T.astype(np.float32))
    com = {
        "xTf": xT.reshape(ND, P, T),
        "xbf": _bf16(np.concatenate([x, np.zeros((1, D), np.float32)], axis=0)),
        "gwT": np.ascontiguousarray(gate_w.T.astype(np.float32)).reshape(ND, P, E),
        "gbias": gate_b.reshape(1, E).astype(np.float32),
        "ones1": np.ones((1, P), np.float32),
        "ut128": np.triu(np.ones((P, P), np.float32)),
        "iota128": np.arange(P, dtype=np.float32).reshape(P, 1),
        "identbf": _bf16(np.eye(P, dtype=np.float32)),
        "initmeta": np.tile(np.array([[float(T), 0.0]], np.float32),
                            (CKMAX, 1)),
    }
    wc = np.zeros((P, NT * NT), np.float32)
    for jp in range(NT):
        for j in range(NT):
            if jp < j:
                wc[:, jp * NT + j] = 1.0
    com["wcum"] = wc
    ej = np.zeros((NT, NT * P), np.float32)
    for j in range(NT):
        ej[j, j * P:(j + 1) * P] = 1.0
    com["ejall"] = ej

    sgp = np.zeros((D, SIP), np.float32)
    sup = np.zeros((D, SIP), np.float32)
    sdp = np.zeros((SIP, D), np.float32)

    in_maps = []
    for c in range(NC):
        m = dict(com)
        oh = np.zeros((P, 4, E), np.float32)
        for k in range(4):
            oh[:, k, assign[c, k]] = 1.0
        m["onehot"] = oh
        for k in range(4):
            e = assign[c, k]
            m[f"wgu{k}"] = _bf16(rearrange(w_gate[e],
                                           "(dc p) (it i) -> it p dc i",
                                           p=P, i=P))
            m[f"wuu{k}"] = _bf16(rearrange(w_up[e],
                                           "(dc p) (it i) -> it p dc i",
                                           p=P, i=P))
            m[f"wdd{k}"] = _wd_swizzle(w_down[e], NIP)
        sgp[:, :SI] = sw_gate[:, c * SI:(c + 1) * SI]
        sup[:, :SI] = sw_up[:, c * SI:(c + 1) * SI]
        sdp[:SI, :] = sw_down[c * SI:(c + 1) * SI, :]
        m["swg"] = _bf16(rearrange(sgp, "(dc p) (it i) -> it p dc i", p=P, i=P))
        m["swu"] = _bf16(rearrange(sup, "(dc p) (it i) -> it p dc i", p=P, i=P))
        m["swd"] = _wd_swizzle(sdp, NSIP)
        in_maps.append(m)
    return in_maps


def _run(inputs, trace=False):
    x = np.asarray(inputs["x"], np.float32)
    gate_w = np.asarray(inputs["gate_w"], np.float32)
    gate_b = np.asarray(inputs["gate_b"], np.float32)
    counts = _routing_counts(x, gate_w, gate_b)
    assign, prof = _assignment(counts)
    if prof not in _program_cache:
        _program_cache[prof] = build_program(prof)
    nc = _program_cache[prof]
    in_maps = _prep_inputs(
        x, gate_w, gate_b,
        np.asarray(inputs["w_gate"], np.float32),
        np.asarray(inputs["w_up"], np.float32),
        np.asarray(inputs["w_down"], np.float32),
        np.asarray(inputs["sw_gate"], np.float32),
        np.asarray(inputs["sw_up"], np.float32),
        np.asarray(inputs["sw_down"], np.float32),
        assign, prof)
    res = run_bass_kernel_spmd(nc, in_maps, core_ids=list(range(NC)),
                               trace=trace)
    out = np.concatenate([res.results[c]["ysh"] for c in range(NC)], axis=0)
    return out.astype(np.float32), res


def kernel(**inputs):
    out, _ = _run(inputs, trace=False)
    return out
